# revision 10
# baseline (speedup 1.0000x reference)
"""LiquidNN Trainium2 kernel.

Strategy: the time recurrence serializes steps, and every recurrent matmul
[32,1024]@[1024,1024] is PE weight-streaming bound (batch 32 << 128), so
data-parallelism over batch buys nothing.  Instead: layer-pipeline across 4
cores (one layer per core, weights SBUF-resident), h handed to the next
stage each step via AllGather; cores 4-7 run a redundant second pipeline
(SPMD symmetry).  Per-core behavior is injected purely via input data:
per-core weights, per-core gather indices (indirect DMA selects the previous
rank's AllGather slice; rank 0 points at a zero region and adds the real x
stream instead), and a per-iteration warmup mask folded into tanh's scale.

Layouts (everything feature-partitioned, batch on the free axis):
  h/x tile [128, 256]: tile[p, 32*c + j] = value[feature 128*c + p, batch j]
  W SBUF [128, 8192]:  w[p, 1024*kc + m] = W[128*kc + p, m]
  out = lhsT.T @ rhs accumulated over kc into PSUM [128, 32] per mc chunk.
"""

import numpy as np

B, T, D, H, L = 32, 512, 1024, 1024, 4
N_RANKS = 4
N_ITER = T + (N_RANKS - 1)  # pipeline skew

# dtype knob for weights/activations in the recurrence
DT_NP = np.float32

_CACHE = {}


def _w_layout(w, dt):
    # [K, M] -> SBUF [128, 8*1024]
    k, m = w.shape
    assert k == 1024 and m == 1024
    return np.ascontiguousarray(
        w.reshape(8, 128, 1024).transpose(1, 0, 2).reshape(128, 8192)
    ).astype(dt)


def _b_tile(v):
    # [1024] -> [128, 256] chunked per mc
    bb = v.reshape(8, 128).T  # [p, mc]
    return np.ascontiguousarray(
        np.broadcast_to(bb[:, :, None], (128, 8, 32)).reshape(128, 256)
    ).astype(np.float32)


def _x_stream(x, n_iter, dt):
    # x [B, T, D] -> [n_iter, 128, 256]
    t = x.shape[1]
    xs = x.transpose(1, 2, 0).reshape(t, 8, 128, B).transpose(0, 2, 1, 3)
    xs = np.ascontiguousarray(xs.reshape(t, 128, 8 * B)).astype(dt)
    if n_iter > t:
        xs = np.concatenate(
            [xs, np.zeros((n_iter - t, 128, 8 * B), dtype=dt)], axis=0
        )
    return xs


def _build(n_iter, dt_np, with_tau):
    import concourse.bass as bass
    import concourse.bacc as bacc
    import concourse.mybir as mybir
    import concourse.tile as tile

    dt_w = mybir.dt.from_np(np.dtype(dt_np))
    f32 = mybir.dt.float32

    nc = bacc.Bacc(
        "TRN2", target_bir_lowering=False, debug=False, num_devices=8
    )

    wi_d = nc.dram_tensor("wi", [128, 8192], dt_w, kind="ExternalInput")
    wh_d = nc.dram_tensor("wh", [128, 8192], dt_w, kind="ExternalInput")
    wo_d = nc.dram_tensor("wo", [128, 8192], dt_w, kind="ExternalInput")
    bt_d = nc.dram_tensor("bt", [128, 256], f32, kind="ExternalInput")
    bo_d = nc.dram_tensor("bo", [128, 256], f32, kind="ExternalInput")
    xs_d = nc.dram_tensor("xs", [n_iter, 128, 256], dt_w, kind="ExternalInput")
    mk_d = nc.dram_tensor("mk", [n_iter, 128, 1], f32, kind="ExternalInput")
    sm_d = nc.dram_tensor("sm", [128, 4], f32, kind="ExternalInput")
    it_d = nc.dram_tensor("it", [128, 256], f32, kind="ExternalInput")
    ic_d = nc.dram_tensor("ic", [128, 256], f32, kind="ExternalInput")
    out_d = nc.dram_tensor("out", [128, 256], f32, kind="ExternalOutput")

    with tile.TileContext(nc) as tc:
        with (
            tc.tile_pool(name="wpool", bufs=1) as wpool,
            tc.tile_pool(name="spool", bufs=3) as spool,
            tc.tile_pool(name="hpool", bufs=2) as hpool,
            tc.tile_pool(name="pspool", bufs=2, space="PSUM") as pspool,
            tc.tile_pool(name="dpool", bufs=2, space="DRAM") as dpool,
        ):
            wi = wpool.tile([128, 8192], dt_w, tag="wi")
            nc.sync.dma_start(wi[:], wi_d.ap())
            wh = wpool.tile([128, 8192], dt_w, tag="wh")
            nc.sync.dma_start(wh[:], wh_d.ap())
            wo = wpool.tile([128, 8192], dt_w, tag="wo")
            nc.sync.dma_start(wo[:], wo_d.ap())
            bt = wpool.tile([128, 256], f32, tag="bt")
            nc.sync.dma_start(bt[:], bt_d.ap())
            bo = wpool.tile([128, 256], f32, tag="bo")
            nc.sync.dma_start(bo[:], bo_d.ap())
            sm = wpool.tile([128, 4], f32, tag="sm")
            nc.sync.dma_start(sm[:], sm_d.ap())
            if with_tau:
                it_t = wpool.tile([128, 256], f32, tag="it")
                nc.sync.dma_start(it_t[:], it_d.ap())
                ic_t = wpool.tile([128, 256], f32, tag="ic")
                nc.sync.dma_start(ic_t[:], ic_d.ap())

            # AllGather arena: rows 0-511 = gathered h tiles (4 ranks).
            arena = dpool.tile([512, 256], dt_w, tag="arena")
            zt = wpool.tile([128, 256], dt_w, tag="zt")
            nc.vector.memset(zt[:], 0.0)
            for k in range(4):
                nc.sync.dma_start(arena[128 * k : 128 * (k + 1), :], zt[:])

            h = hpool.tile([128, 256], dt_w, tag="h")
            nc.vector.memset(h[:], 0.0)

            mult, add = mybir.AluOpType.mult, mybir.AluOpType.add

            for n in range(n_iter):
                xa = spool.tile([128, 1024], dt_w, tag="xa")
                for q in range(4):
                    nc.sync.dma_start(
                        xa[:, 256 * q : 256 * (q + 1)],
                        arena[128 * q : 128 * (q + 1), :],
                    )
                xt = spool.tile([128, 256], dt_w, tag="xt")
                nc.sync.dma_start(xt[:], xs_d.ap()[n])
                # xin = xs + sum_q arena_block_q * sm[q]  (sm one-hot = prev
                # rank's block; all-zero on rank 0, which uses the x stream)
                xin = spool.tile([128, 256], dt_w, tag="xin")
                t0 = spool.tile([128, 256], dt_w, tag="t0")
                t1 = spool.tile([128, 256], dt_w, tag="t1")
                nc.vector.scalar_tensor_tensor(
                    t0[:], xa[:, 0:256], sm[:, 0:1], xt[:], mult, add
                )
                nc.vector.scalar_tensor_tensor(
                    t1[:], xa[:, 256:512], sm[:, 1:2], t0[:], mult, add
                )
                nc.vector.scalar_tensor_tensor(
                    t0[:], xa[:, 512:768], sm[:, 2:3], t1[:], mult, add
                )
                nc.vector.scalar_tensor_tensor(
                    xin[:], xa[:, 768:1024], sm[:, 3:4], t0[:], mult, add
                )
                mk = spool.tile([128, 1], f32, tag="mk")
                nc.sync.dma_start(mk[:], mk_d.ap()[n])

                ps = pspool.tile([128, 256], f32, tag="ps")
                for mc in range(8):
                    pslice = ps[:, 32 * mc : 32 * mc + 32]
                    for kc in range(8):
                        nc.tensor.matmul(
                            pslice,
                            wh[:, 1024 * kc + 128 * mc : 1024 * kc + 128 * mc + 128],
                            h[:, 32 * kc : 32 * kc + 32],
                            start=(kc == 0),
                            stop=False,
                        )
                    for kc in range(8):
                        nc.tensor.matmul(
                            pslice,
                            wi[:, 1024 * kc + 128 * mc : 1024 * kc + 128 * mc + 128],
                            xin[:, 32 * kc : 32 * kc + 32],
                            start=False,
                            stop=(kc == 7),
                        )
                nc.vector.tensor_add(ps[:], ps[:], bt[:])
                h_new = hpool.tile([128, 256], dt_w, tag="h")
                nc.scalar.activation(
                    h_new[:],
                    ps[:],
                    mybir.ActivationFunctionType.Tanh,
                    scale=mk[:, 0:1],
                )
                if with_tau:
                    # h = h*(1 - 1/tau) + dx*(1/tau); dx currently in h_new
                    hm = hpool.tile([128, 256], dt_w, tag="hm")
                    nc.vector.tensor_mul(hm[:], h[:], ic_t[:])
                    nc.vector.tensor_mul(h_new[:], h_new[:], it_t[:])
                    nc.vector.tensor_add(h_new[:], h_new[:], hm[:])

                agin = dpool.tile([128, 256], dt_w, tag="agin")
                nc.sync.dma_start(agin[:], h_new[:])
                nc.gpsimd.collective_compute(
                    "AllGather",
                    mybir.AluOpType.bypass,
                    ins=[agin[:]],
                    outs=[arena[:]],
                    replica_groups=[[0, 1, 2, 3], [4, 5, 6, 7]],
                )
                h = h_new

            ps2 = pspool.tile([128, 256], f32, tag="ps")
            for mc in range(8):
                pslice = ps2[:, 32 * mc : 32 * mc + 32]
                for kc in range(8):
                    nc.tensor.matmul(
                        pslice,
                        wo[:, 1024 * kc + 128 * mc : 1024 * kc + 128 * mc + 128],
                        h[:, 32 * kc : 32 * kc + 32],
                        start=(kc == 0),
                        stop=(kc == 7),
                    )
            nc.vector.tensor_add(ps2[:], ps2[:], bo[:])
            osb = spool.tile([128, 256], f32, tag="osb")
            nc.vector.tensor_copy(osb[:], ps2[:])
            nc.sync.dma_start(out_d.ap(), osb[:])

    nc.compile()
    return nc


def _in_maps(x, W_in, b_in, W_h, b_h, tau, W_out, b_out, n_iter, dt_np, with_tau):
    t_real = x.shape[1]
    xs_real = _x_stream(np.asarray(x), n_iter, dt_np)
    xs_zero = np.zeros_like(xs_real)
    wo = _w_layout(np.asarray(W_out), dt_np)
    bo = _b_tile(np.asarray(b_out))
    maps = []
    for c in range(8):
        r = c % 4
        sm = np.zeros((128, 4), dtype=np.float32)
        if r > 0:
            sm[:, r - 1] = 1.0
        mk = np.zeros((n_iter, 128, 1), dtype=np.float32)
        mk[r:] = 1.0
        m = {
            "wi": _w_layout(np.asarray(W_in[r]), dt_np),
            "wh": _w_layout(np.asarray(W_h[r]), dt_np),
            "wo": wo,
            "bt": _b_tile(np.asarray(b_in[r]) + np.asarray(b_h[r])),
            "bo": bo,
            "xs": xs_real if r == 0 else xs_zero,
            "mk": mk,
            "sm": sm,
            "it": _b_tile(1.0 / np.asarray(tau[r], dtype=np.float64)),
            "ic": _b_tile(1.0 - 1.0 / np.asarray(tau[r], dtype=np.float64)),
        }
        maps.append(m)
    return maps


def _unshard_out(res):
    # [128, 256] -> [32, 1024]
    return np.ascontiguousarray(
        res.reshape(128, 8, 32).transpose(2, 1, 0).reshape(32, 1024)
    ).astype(np.float32)


def run_hw(x, W_in, b_in, W_h, b_h, tau, W_out, b_out, n_iter=None, trace=False):
    from concourse import bass_utils

    if n_iter is None:
        n_iter = x.shape[1] + (N_RANKS - 1)
    with_tau = not np.allclose(np.asarray(tau), 1.0)
    key = (n_iter, np.dtype(DT_NP).name, with_tau)
    if key not in _CACHE:
        _CACHE[key] = _build(n_iter, DT_NP, with_tau)
    nc = _CACHE[key]
    maps = _in_maps(
        x, W_in, b_in, W_h, b_h, tau, W_out, b_out, n_iter, DT_NP, with_tau
    )
    res = bass_utils.run_bass_kernel_spmd(
        nc, maps, core_ids=list(range(8)), trace=trace
    )
    out = _unshard_out(res.results[3]["out"])
    return out, res


def kernel(x, W_in, b_in, W_h, b_h, tau, W_out, b_out):
    out, _ = run_hw(x, W_in, b_in, W_h, b_h, tau, W_out, b_out)
    return out
